# revision 69
# baseline (speedup 1.0000x reference)
"""GraphSAGE 3-layer + output projection on 8 Trainium2 NeuronCores.

Sharding: nodes (and dst-partitioned edges) split across 8 cores, 1280
nodes/core (N padded 10000->10240). Per layer, activations live
replicated in DRAM (fp8, piecewise AllGather layout); each core batch-
gathers its edges' source rows with SWDGE dma_gather (<=1024 idxs per
call), segment-sums them on the TensorEngine via host-precomputed fp8
one-hot DoubleRow matmuls (256 edges/matmul, 0.5 cyc/row), scales by
1/deg, transposes, and applies lin_l/lin_r as bf16 matmuls. The inter-
layer AllGather is split into three pieces (512/512/256 nodes/core);
each dst tile's gather is split into three matching waves so next-layer
gather work overlaps in-flight collectives. Wave A+B partials flush to
bf16 scratch and are merged back via an identity matmul when wave C's
piece has landed. Trailing pad slots use -1 gather indices (skipped
descriptors); xs buffers are memzeroed once so skipped slots never feed
NaN garbage into the PE.
"""
import sys, types, ctypes, contextlib

import numpy as np


def _install_ntff_hook():
    # antenv.axon_hooks is missing in this image; provide it so
    # bass_utils trace=True can profile via libaxon_pjrt.so.
    if "antenv.axon_hooks" in sys.modules:
        return
    try:
        import antenv  # noqa: F401
    except ImportError:
        return
    mod = types.ModuleType("antenv.axon_hooks")
    state = {"hook": None}
    mod.set_axon_ntff_profile_hook = lambda h: state.__setitem__("hook", h)
    mod.get_axon_ntff_profile_hook = lambda: state["hook"]
    sys.modules["antenv.axon_hooks"] = mod
    try:
        lib = ctypes.CDLL('/opt/axon/libaxon_pjrt.so')
    except OSError:
        return
    if not hasattr(lib, "axon_start_nrt_profile"):
        return
    lib.axon_start_nrt_profile.argtypes = [ctypes.POINTER(ctypes.c_int64), ctypes.c_size_t]
    lib.axon_start_nrt_profile.restype = ctypes.c_int64
    lib.axon_stop_nrt_profile.argtypes = [ctypes.c_char_p]
    lib.axon_stop_nrt_profile.restype = ctypes.c_int64

    @contextlib.contextmanager
    def _hook(output_dir, device_ids):
        import jax
        jax.devices()
        if device_ids:
            ids = (ctypes.c_int64 * len(device_ids))(*device_ids)
            rc = lib.axon_start_nrt_profile(ids, len(device_ids))
        else:
            rc = lib.axon_start_nrt_profile(None, 0)
        if rc != 0:
            raise RuntimeError(f"axon_start_nrt_profile rc={rc}")
        try:
            yield
        finally:
            n = lib.axon_stop_nrt_profile(str(output_dir).encode())
            print(f"profile: {n} file(s) written to {output_dir}", file=sys.stderr)

    state["hook"] = _hook


_install_ntff_hook()

import concourse.bass2jax as _b2j
_orig_cc_hook = _b2j.neuronx_cc_hook
def _dbg_cc_hook(*a, **kw):
    try:
        return _orig_cc_hook(*a, **kw)
    except BaseException:
        import traceback
        traceback.print_exc()
        raise
_b2j.neuronx_cc_hook = _dbg_cc_hook

import concourse.bass as bass
import concourse.tile as tile
from concourse import mybir, bacc
from concourse.bass_utils import run_bass_kernel_spmd
from concourse.masks import make_identity

F32 = mybir.dt.float32
F32R = mybir.dt.float32r
BF16 = mybir.dt.bfloat16
I32 = mybir.dt.int32
I16 = mybir.dt.int16
FP8 = mybir.dt.float8e4

N, D, H, O = 10000, 512, 512, 128
C = 8              # cores
NP = 10240         # padded node count
NCORE = NP // C    # 1280 nodes per core
NT = NCORE // 128  # 10 dst tiles per core
GROUPS = [(0, 512), (512, 512), (1024, 256)]  # dense node groups


def _host_prep(x, edge_index):
    src = np.asarray(edge_index[0], dtype=np.int64)
    dst = np.asarray(edge_index[1], dtype=np.int64)
    deg = np.bincount(dst, minlength=NP).astype(np.float64)
    invdeg = (1.0 / np.maximum(deg, 1.0)).astype(np.float32)

    order = np.argsort(dst, kind="stable")
    src_s = src[order]
    dst_s = dst[order]

    # piece-wise AllGather layout (all layers; x_full0 is pre-remapped):
    # pieces of 512/512/256 nodes per core
    allnodes = np.arange(NP, dtype=np.int64)
    cc, loc = allnodes // NCORE, allnodes % NCORE
    remap = np.where(
        loc < 512, cc * 512 + loc,
        np.where(loc < 1024, 4096 + cc * 512 + (loc - 512),
                 8192 + cc * 256 + (loc - 1024))).astype(np.int32)

    # three gather waves per dst tile, one per AllGather piece of the
    # previous layer; indices are region-local so the gather src AP pins
    # the exact DRAM region dep
    bnds = np.searchsorted(dst_s, np.arange(0, NP + 1, 128))
    wav_edges = {}
    cntW = np.zeros((3, C, NT), np.int64)
    for c in range(C):
        for t in range(NT):
            g = c * NT + t
            lo, hi = bnds[g], bnds[g + 1]
            r = remap[src_s[lo:hi]]
            doff = (dst_s[lo:hi] - g * 128).astype(np.int16)
            for w, (rlo, rhi) in enumerate(((0, 4096), (4096, 8192), (8192, NP))):
                m = (r >= rlo) & (r < rhi)
                wav_edges[w, c, t] = ((r[m] - rlo).astype(np.int16), doff[m])
                cntW[w, c, t] = int(m.sum())

    TW = np.maximum(np.ceil(np.maximum(cntW, 1) / 128).astype(np.int64).max(axis=1), 1)
    T = TW.sum(axis=0)                    # [NT] blocks per tile
    bases = np.concatenate([[0], np.cumsum(T)])[:-1]
    wbase = np.stack([bases, bases + TW[0], bases + TW[0] + TW[1]])  # [3, NT]
    ST = int(T.sum())

    # vmax: max valid idxs over cores per (wave, tile); slots beyond get
    # idx -1 (trailing, identical across cores) so the gather skips them
    vmax = np.maximum(cntW.max(axis=1), 1)            # [3, NT]

    srcidx2 = np.zeros((C, 128, ST), np.int32)
    dstoff = np.full((C, 128, ST), 255, np.int16)
    for c in range(C):
        for t in range(NT):
            for w in range(3):
                idxv, doffv = wav_edges[w, c, t]
                b = int(wbase[w, t])
                n = len(idxv)
                if n:
                    e = np.arange(n)
                    srcidx2[c, e % 128, b + e // 128] = idxv
                    dstoff[c, e % 128, b + e // 128] = doffv
                e = np.arange(int(vmax[w, t]), int(TW[w, t]) * 128)
                srcidx2[c, e % 128, b + e // 128] = -1

    x_pad = np.zeros((NP, D), np.float32)
    x_pad[:N] = np.asarray(x, dtype=np.float32)

    invdeg_sb = np.empty((C, 128, NT), np.float32)
    for c in range(C):
        invdeg_sb[c] = invdeg[c * NCORE:(c + 1) * NCORE].reshape(NT, 128).T

    import ml_dtypes
    xT0 = np.empty((C, 128, 4, NCORE), ml_dtypes.bfloat16)
    for c in range(C):
        xT0[c] = x_pad[c * NCORE:(c + 1) * NCORE].reshape(NCORE, 4, 128).transpose(2, 1, 0)

    # x_full0 pre-remapped into the piecewise AllGather layout (fp8)
    x0r = np.zeros((NP, D), ml_dtypes.float8_e4m3)
    x0r[remap] = x_pad.astype(ml_dtypes.float8_e4m3)

    # host-precomputed one-hot cache: oh[p, i, d] = (dstoff[p, i] == d), fp8
    ohs = (dstoff[:, :, :, None] ==
           np.arange(128, dtype=np.int16)[None, None, None, :]).astype(
               ml_dtypes.float8_e4m3)                      # [C, 128, ST, 128]

    # dma_gather idx arrays: int16, 16-partition wrap, replicated x8.
    # slot j of tile t -> (partition j%128, block j//128); unwrapped[j] =
    # idxs[j%16, j//16], so idx16[p, b*8 + s] = srcidx[c, (s*16+p)%128, b + (s*16+p)//128]
    def _wrap16(arr):
        out = np.zeros((C, 128, ST * 8), np.int16)
        for c in range(C):
            for t in range(NT):
                b, ne = int(bases[t]), int(T[t])
                flat = arr[c][:, b:b + ne]                 # [128 part, ne blocks]
                j = np.arange(ne * 128)
                vals = flat[j % 128, j // 128]             # slot-ordered
                w = vals.reshape(ne * 8, 16).T             # [16, ne*8]
                out[c, :, b * 8:(b + ne) * 8] = np.tile(w, (8, 1))
        return out

    srcidx216 = _wrap16(srcidx2)

    return x0r, srcidx216, ohs, invdeg_sb, xT0, TW, wbase, vmax, ST


def _wsb(w):
    # [K, M] -> SBUF layout [128, K/128, M], bf16
    import ml_dtypes
    w = np.asarray(w, np.float32)
    return np.ascontiguousarray(
        w.reshape(w.shape[0] // 128, 128, w.shape[1]).transpose(1, 0, 2)
    ).astype(ml_dtypes.bfloat16)


def _bsb(b):
    # [M] -> [128, M/128]
    b = np.asarray(b, np.float32)
    return np.ascontiguousarray(b.reshape(b.shape[0] // 128, 128).T)


XSBUFS = 8


def _build_program(TW, wbase, vmax, ST):
    nc = bacc.Bacc(None, target_bir_lowering=False, debug=False, num_devices=C,
                   num_swdge_queues=4)

    TWMAX = int(TW.max())
    x0_d = nc.declare_dram_parameter("x_full0", [NP, D], FP8, isOutput=False)
    srcidx2_d = nc.declare_dram_parameter("srcidx2", [128, ST * 8], I16, isOutput=False)
    ohs_d = nc.declare_dram_parameter("ohs", [128, ST, 128], FP8, isOutput=False)
    invdeg_d = nc.declare_dram_parameter("invdeg", [128, NT], F32, isOutput=False)
    xT0_d = nc.declare_dram_parameter("xT0", [128, 4, NCORE], BF16, isOutput=False)
    w_d = {}
    for l in range(3):
        w_d[f"wl{l}"] = nc.declare_dram_parameter(f"wl{l}", [128, 4, H], BF16, isOutput=False)
        w_d[f"wr{l}"] = nc.declare_dram_parameter(f"wr{l}", [128, 4, H], BF16, isOutput=False)
        w_d[f"b{l}"] = nc.declare_dram_parameter(f"b{l}", [128, 4], F32, isOutput=False)
    wout_d = nc.declare_dram_parameter("wout", [128, 4, O], BF16, isOutput=False)
    bout_d = nc.declare_dram_parameter("bout", [128, 1], F32, isOutput=False)
    out_d = nc.declare_dram_parameter("out", [NCORE, O], F32, isOutput=True)

    xg = [None, nc.dram_tensor("xg1", [NP, D], FP8, addr_space="Shared"),
          nc.dram_tensor("xg2", [NP, D], FP8, addr_space="Shared")]
    xc = [None, nc.dram_tensor("xc1", [NCORE, D], FP8),
          nc.dram_tensor("xc2", [NCORE, D], FP8)]

    with tile.TileContext(nc) as tc:
        with tc.tile_pool(name="const", bufs=1) as constp, \
             tc.tile_pool(name="xT", bufs=2) as xTp, \
             tc.tile_pool(name="aggT", bufs=1) as aggTp, \
             tc.tile_pool(name="xs", bufs=XSBUFS) as xsp, \
             tc.tile_pool(name="aggP", bufs=10) as aggPp, \
             tc.tile_pool(name="agg", bufs=2) as aggp, \
             tc.tile_pool(name="xnm", bufs=3) as xnmp, \
             tc.tile_pool(name="pa", bufs=2, space="PSUM") as pap, \
             tc.tile_pool(name="pt", bufs=2, space="PSUM") as ptp, \
             tc.tile_pool(name="pd", bufs=2, space="PSUM") as pdp:

            # ---- load constants ----
            srcidx2_sb = constp.tile([128, ST * 8], I16)
            nc.sync.dma_start(srcidx2_sb[:], srcidx2_d[:])
            ohs_sb = constp.tile([128, ST, 128], FP8)
            nc.sync.dma_start(ohs_sb[:], ohs_d[:])
            invdeg_sb = constp.tile([128, NT], F32)
            nc.sync.dma_start(invdeg_sb[:], invdeg_d[:])
            wsb = {}
            for l in range(3):
                for nm in (f"wl{l}", f"wr{l}"):
                    wsb[nm] = constp.tile([128, 4, H], BF16, name=nm)
                    nc.sync.dma_start(wsb[nm][:], w_d[nm][:])
                wsb[f"b{l}"] = constp.tile([128, 4], F32, name=f"bsb{l}")
                nc.sync.dma_start(wsb[f"b{l}"][:], w_d[f"b{l}"][:])
            wout_sb = constp.tile([128, 4, O], BF16)
            nc.sync.dma_start(wout_sb[:], wout_d[:])
            bout_sb = constp.tile([128, 1], F32)
            nc.sync.dma_start(bout_sb[:], bout_d[:])

            ident = constp.tile([128, 128], F32)
            make_identity(nc, ident[:])
            identb = constp.tile([128, 128], BF16)
            make_identity(nc, identb[:])

            xT_cur = xTp.tile([128, 4, NCORE], BF16)
            nc.sync.dma_start(xT_cur[:], xT0_d[:])

            # memzero the xs gather buffers once: trailing -1 gather idxs skip
            # the write, and stale SBUF garbage must not be NaN (NaN*0 = NaN
            # in the one-hot matmul)
            for _ in range(XSBUFS):
                z = xsp.tile([128, TWMAX, D], FP8, name="xs")
                nc.vector.memset(z[:], 0.0)

            qn = [0]
            pending_cc = [None]

            for l in range(3):
                xsrc = x0_d if l == 0 else xg[l]
                regions = (xsrc[0:4096, :], xsrc[4096:8192, :], xsrc[8192:NP, :])
                aggT = aggTp.tile([128, 4, NCORE], BF16)
                xT_next = xTp.tile([128, 4, NCORE], BF16)
                wl, wr, bb = wsb[f"wl{l}"], wsb[f"wr{l}"], wsb[f"b{l}"]
                aggPs = {}

                def do_gather_mm(w, t, pa, first, close):
                    # gathers (chunks of <=8 blocks; 1024-idx HW limit) then
                    # one-hot DoubleRow matmuls accumulating into pa
                    b = int(wbase[w, t])
                    ne = int(TW[w, t])
                    nv = int(vmax[w, t])       # valid idxs (then trailing -1s)
                    xs = xsp.tile([128, TWMAX, D], FP8, name="xs")
                    off = 0
                    while off < ne:
                        ch = min(8, ne - off)
                        nc.gpsimd.dma_gather(
                            xs[:, off:off + ch, :], regions[w],
                            srcidx2_sb[:, (b + off) * 8:(b + off + ch) * 8],
                            num_idxs=ch * 128,
                            num_idxs_reg=min(ch * 128, nv - off * 128),
                            elem_size=D, queue_num=qn[0] % 4)
                        qn[0] += 1
                        off += ch
                    e = 0
                    while e < ne:
                        if ne - e >= 2:
                            nc.tensor.matmul(
                                pa[:], lhsT=ohs_sb[:, b + e:b + e + 2, :],
                                rhs=xs[:, e:e + 2, :],
                                start=first and (e == 0), stop=close and (e + 2 == ne),
                                perf_mode=mybir.MatmulPerfMode.DoubleRow)
                            e += 2
                        else:
                            nc.tensor.matmul(
                                pa[:], lhsT=ohs_sb[:, b + e, :], rhs=xs[:, e, :],
                                start=first and (e == 0), stop=close and (e == ne - 1))
                            e += 1

                def do_wave1(t):
                    pa = pap.tile([128, D], F32, name="pa")
                    do_gather_mm(0, t, pa, first=True, close=False)
                    do_gather_mm(1, t, pa, first=False, close=True)
                    aggP = aggPp.tile([128, D], BF16, name="aggP")
                    nc.scalar.activation(
                        aggP[:], pa[:], mybir.ActivationFunctionType.Copy)
                    aggPs[t] = aggP

                def do_wave2(t):
                    pa = pap.tile([128, D], F32, name="pa")
                    do_gather_mm(2, t, pa, first=True, close=False)
                    nc.tensor.matmul(
                        pa[:], lhsT=identb[:], rhs=aggPs[t][:],
                        start=False, stop=True)
                    agg = aggp.tile([128, D], F32, name="agg")
                    nc.scalar.activation(
                        agg[:], pa[:], mybir.ActivationFunctionType.Copy,
                        scale=invdeg_sb[:, t:t + 1])
                    for k in range(4):
                        pt = ptp.tile([128, 128], F32, name="pt")
                        nc.tensor.transpose(pt[:], agg[:, k * 128:(k + 1) * 128], ident[:])
                        nc.vector.tensor_copy(aggT[:, k, t * 128:(t + 1) * 128], pt[:])

                def do_dense_group(goff, gsz):
                    for m in range(4):
                        pd = pdp.tile([128, 768], F32, name="pd")
                        for o in range(0, gsz, 512):
                            w_ = min(512, gsz - o)
                            for k in range(4):
                                nc.tensor.matmul(
                                    pd[:, o:o + w_],
                                    lhsT=wl[:, k, m * 128:(m + 1) * 128],
                                    rhs=aggT[:, k, goff + o:goff + o + w_],
                                    start=(k == 0), stop=False)
                            for k in range(4):
                                nc.tensor.matmul(
                                    pd[:, o:o + w_],
                                    lhsT=wr[:, k, m * 128:(m + 1) * 128],
                                    rhs=xT_cur[:, k, goff + o:goff + o + w_],
                                    start=False, stop=(k == 3))
                            nc.scalar.activation(
                                xT_next[:, m, goff + o:goff + o + w_], pd[:, o:o + w_],
                                mybir.ActivationFunctionType.Relu,
                                bias=bb[:, m:m + 1])
                    if l < 2:
                        for t in range(goff // 128, (goff + gsz) // 128):
                            xnm = xnmp.tile([128, D], FP8, name="xnm")
                            for k in range(4):
                                pt = ptp.tile([128, 128], BF16, name="pt")
                                nc.tensor.transpose(
                                    pt[:], xT_next[:, k, t * 128:(t + 1) * 128],
                                    identb[:])
                                nc.vector.tensor_copy(xnm[:, k * 128:(k + 1) * 128], pt[:])
                            nc.sync.dma_start(xc[l + 1][t * 128:(t + 1) * 128, :], xnm[:])

                # per-piece waves let each gather wait only on the AllGather
                # piece it actually reads; dense groups and their collectives
                # fire as soon as their four tiles complete
                for t in range(4):
                    do_wave1(t)
                    if t == 1 and pending_cc[0] is not None:
                        pending_cc[0]()
                        pending_cc[0] = None
                for t in range(4):
                    do_wave2(t)
                do_dense_group(0, 512)
                if l < 2:
                    nc.gpsimd.collective_compute(
                        "AllGather", mybir.AluOpType.bypass,
                        replica_groups=[list(range(C))],
                        ins=[xc[l + 1][0:512, :]], outs=[xg[l + 1][0:4096, :]])
                for t in range(4, 8):
                    do_wave1(t)
                for t in range(4, 8):
                    do_wave2(t)
                do_dense_group(512, 512)
                if l < 2:
                    nc.gpsimd.collective_compute(
                        "AllGather", mybir.AluOpType.bypass,
                        replica_groups=[list(range(C))],
                        ins=[xc[l + 1][512:1024, :]], outs=[xg[l + 1][4096:8192, :]])
                for t in range(8, 10):
                    do_wave1(t)
                for t in range(8, 10):
                    do_wave2(t)
                do_dense_group(1024, 256)

                def fire_piece3(l=l):
                    nc.gpsimd.collective_compute(
                        "AllGather", mybir.AluOpType.bypass,
                        replica_groups=[list(range(C))],
                        ins=[xc[l + 1][1024:1280, :]], outs=[xg[l + 1][8192:10240, :]])

                if l < 2:
                    pending_cc[0] = fire_piece3
                xT_cur = xT_next

            # final projection x3 @ w_out + b_out  (feat-major out, O=128)
            for goff, gsz in GROUPS:
                pd = pdp.tile([128, 512], F32)
                for k in range(4):
                    nc.tensor.matmul(
                        pd[:, :gsz],
                        lhsT=wout_sb[:, k, :],
                        rhs=xT_cur[:, k, goff:goff + gsz],
                        start=(k == 0), stop=(k == 3))
                oT = aggp.tile([128, 512], F32)
                nc.scalar.activation(
                    oT[:, :gsz], pd[:, :gsz],
                    mybir.ActivationFunctionType.Identity, bias=bout_sb[:, 0:1])
                for tt in range(gsz // 128):
                    t = goff // 128 + tt
                    pt = ptp.tile([128, 128], F32)
                    nc.tensor.transpose(pt[:], oT[:, tt * 128:(tt + 1) * 128], ident[:])
                    onm = xnmp.tile([128, O], F32)
                    nc.vector.tensor_copy(onm[:], pt[:])
                    nc.sync.dma_start(out_d[t * 128:(t + 1) * 128, :], onm[:])

    nc.compile()
    return nc


def _run(inputs, trace=False):
    x = inputs["x"]
    edge_index = inputs["edge_index"]
    x0r, srcidx216, ohs, invdeg_sb, xT0, TW, wbase, vmax, ST = _host_prep(x, edge_index)
    nc = _build_program(TW, wbase, vmax, ST)

    shared = {
        "x_full0": x0r,
        "wout": _wsb(inputs["w_out"]),
        "bout": np.asarray(inputs["b_out"], np.float32).reshape(128, 1),
    }
    for l in range(3):
        shared[f"wl{l}"] = _wsb(inputs[f"w_l{l}"])
        shared[f"wr{l}"] = _wsb(inputs[f"w_r{l}"])
        shared[f"b{l}"] = _bsb(inputs[f"b_l{l}"])

    in_maps = []
    for c in range(C):
        m = dict(shared)
        m["srcidx2"] = np.ascontiguousarray(srcidx216[c])
        m["ohs"] = np.ascontiguousarray(ohs[c])
        m["invdeg"] = np.ascontiguousarray(invdeg_sb[c])
        m["xT0"] = np.ascontiguousarray(xT0[c])
        in_maps.append(m)

    res = run_bass_kernel_spmd(nc, in_maps, list(range(C)), trace=trace)
    out = np.concatenate([res.results[c]["out"] for c in range(C)], axis=0)[:N]
    return out.astype(np.float32), res


def kernel(**inputs):
    out, _ = _run(inputs, trace=False)
    return out


def kernel_timed(**inputs):
    out, res = _run(inputs, trace=True)
    return out, res

